# revision 33
# baseline (speedup 1.0000x reference)
"""HNet energy-via-edge-matching kernel for 8 Trainium2 NeuronCores.

Math (matches the reference exactly, in exact integer arithmetic):
  temp[i,e] = 2*na[i, idx0[e]] + na[i, idx1[e]]          in {0,1,2,3}
  es = code[temp], code = [NOR=2, NCONV=3, NIMPL=5, AND=9]
  filter keeps es values in edge_type_filter, else NULL=0
  energies[i,j] = #{e: L[j,e]==es'[i,e] or L[j,e]==0}
               = null_count[j] + sum_{v kept} (temp==tmap[v]) . (L==v)
  output = energies - min(energies)

Device decomposition per core (4 point-groups x 2 cmp-groups):
  phase 1: any 128-edge chunk touches <=128 distinct endpoint nodes per
    side, so the host gathers those node rows (G, [128,2,P] per chunk)
    and builds one-hot weights (W, [128,2,128]: 2*onehot(idx0) stacked
    with onehot(idx1)); ONE fp8 DoubleRow matmul per chunk then yields
    temp (K=256 contraction instead of K=1024).
  phase 2: K-outer streaming. Per 512-col cmp tile (nt) the kernel
    sweeps edge-chunk pairs; for each chunk the two kept values are
    packed as the two DoubleRow planes (A_v0,A_v1) x (B_v0,B_v1), so a
    single DR matmul accumulates sum_v A_v^T B_v. The first nt pass is
    software-pipelined with phase 1 (per-chunk-pair tiles carry the
    dependencies), so tensor work streams continuously.
  null_count is added as an exact f32 row-broadcast on DVE during the
    PSUM->SBUF copy; per-tile minima are reduced on DVE.
Host only: input staging/layout (edge-chunk node gather, one-hot W,
  transposes, fp8 casts, null counts), global min of per-core mins,
  final subtract during unshard (elementwise, exact fp32 arithmetic).
"""

import numpy as np
import ml_dtypes

import concourse.bacc as bacc
import concourse.mybir as mybir
from concourse.tile import TileContext
from concourse.bass_utils import run_bass_kernel_spmd

# ---- problem constants (hardcoded from spec) ----
N_PTS, N_NODES, N_EDGES, N_CMP = 2048, 1024, 8192, 4096
PGROUPS, CGROUPS = 4, 2          # 8 cores = 4 point-groups x 2 cmp-groups
P = N_PTS // PGROUPS             # 512 points per core
C = N_CMP // CGROUPS             # 2048 cmp columns per core
ECHUNKS = N_EDGES // 128         # 64 edge chunks of 128
KPAIRS = ECHUNKS // 2            # 32 chunk pairs
QUADS = ECHUNKS // 4             # 16 chunk quads (LT DMA granularity)
NTILES = C // 512                # 4 cmp tiles of 512 per core
MTILES = P // 128                # 4 point chunks of 128 per core
LAG = 2                          # phase-2 consumption lag (chunk pairs)

FP8 = mybir.dt.float8e4
F32 = mybir.dt.float32
NP_FP8 = ml_dtypes.float8_e4m3
DR = mybir.MatmulPerfMode.DoubleRow
EQ = mybir.AluOpType.is_equal
RELU = mybir.ActivationFunctionType.Relu

_CODE2TEMP = {2: 0, 3: 1, 5: 2, 9: 3}   # EDG code value -> temp index
TMAX, LMAX = 3, 9

_nc_cache: dict = {}


def _build_nc(pairs):
    """Build the SPMD Bass program. pairs = ((tv0, lv0), (tv1, lv1))."""
    assert len(pairs) == 2
    nc = bacc.Bacc(None, enable_partition_id=False)
    # pre-tiled inputs (host lays out so every DMA is per-partition dense):
    #   W  : [128, ECHUNKS*2*128]  [k, (ec, side, el)] one-hot weights
    #   G  : [KPAIRS, 128, 2*2*P]  [kk][k][(c, side, p)] gathered na rows
    #   LT : [NTILES, QUADS, 128, 4*512]  [nt][q][ki][(c4, j)]
    #   NB : [128, C] f32 null counts replicated across partitions
    W = nc.dram_tensor("W", [128, ECHUNKS * 2 * 128], FP8,
                       kind="ExternalInput")
    G = nc.dram_tensor("G", [KPAIRS, 128, 2 * 2 * P], FP8,
                       kind="ExternalInput")
    LT = nc.dram_tensor("LT", [NTILES, QUADS, 128, 4 * 512], FP8,
                        kind="ExternalInput")
    NB = nc.dram_tensor("NB", [128, C], F32, kind="ExternalInput")
    en = nc.dram_tensor("en", [P, C], F32, kind="ExternalOutput")

    (tv0, lv0), (tv1, lv1) = pairs

    with TileContext(nc) as tc:
        with (
            tc.tile_pool(name="const", bufs=1) as const_pool,
            tc.tile_pool(name="g", bufs=4) as g_pool,
            tc.tile_pool(name="lt", bufs=6) as lt_pool,
            tc.tile_pool(name="b", bufs=10) as b_pool,
            tc.tile_pool(name="a", bufs=1) as a_pool,
            tc.tile_pool(name="out", bufs=6) as out_pool,
            tc.tile_pool(name="ep", bufs=4, space="PSUM") as ep_pool,
            tc.tile_pool(name="tp", bufs=2, space="PSUM") as tp_pool,
        ):
            bias_tiles = {}

            def bias_ap(v):
                v = float(v)
                if v not in bias_tiles:
                    t = const_pool.tile([128, 1], F32,
                                        name=f"bias{len(bias_tiles)}",
                                        tag=f"bias{len(bias_tiles)}")
                    nc.any.memset(t[:], v)
                    bias_tiles[v] = t
                return bias_tiles[v][:]

            # W resident in 8 slice tiles; slice 0 DMAed up front, the
            # rest interleaved into the kk loop so LT/G aren't starved
            wq = ECHUNKS // 8
            w_tiles = [const_pool.tile([128, wq, 2, 128], FP8,
                                       name=f"w{i}", tag=f"w{i}")
                       for i in range(8)]

            def emit_w_dma(i, eng=None):
                (eng or nc.sync).dma_start(
                    out=w_tiles[i][:],
                    in_=W[:, i * wq * 256:(i + 1) * wq * 256])

            # startup DMAs fan out across engine queues: each engine can
            # only dispatch after its own preamble, so serializing these on
            # sync costs ~0.6us apiece on the critical path.  W0 leads on
            # sync (it gates the first LDWEIGHTS), LT00 next (gates the
            # first B mask); G0/LT01 ride gpsimd's SWDGE path; the scalar
            # queue stays clear so its ACT table load runs immediately.
            emit_w_dma(0)
            lt_pre = []
            for q, eng in ((0, nc.sync), (1, nc.gpsimd)):
                t = lt_pool.tile([128, 4, 512], FP8, name=f"lt_pre{q}",
                                 tag="lt")
                eng.dma_start(out=t[:], in_=LT[0, q])
                lt_pre.append(t)
            g_pre = g_pool.tile([128, 2, 2, P], FP8, name="g_pre", tag="g")
            nc.gpsimd.dma_start(out=g_pre[:], in_=G[0])
            nb_sb = const_pool.tile([128, C], F32, tag="nb")
            # touch the activation table so the lazy ACT_TABLE_LOAD (1.3us)
            # runs during the fill window, not before the first real A mask
            warm = const_pool.tile([128, 1], F32, tag="warm")
            nc.scalar.activation(warm[:], bias_ap(0.0), RELU,
                                 bias=bias_ap(0.0), scale=1.0)

            def a_mask(out_ap, in_ap, tv, engine):
                # out = (in == tv), exact on the {0,1,2,3} temp alphabet
                if engine == "act":
                    if tv == TMAX:
                        nc.scalar.activation(out_ap, in_ap, RELU,
                                             bias=bias_ap(1 - tv), scale=1.0)
                    elif tv == 0:
                        nc.scalar.activation(out_ap, in_ap, RELU,
                                             bias=bias_ap(1), scale=-1.0)
                    else:
                        raise ValueError(tv)
                else:
                    nc.vector.tensor_scalar(out=out_ap, in0=in_ap,
                                            scalar1=float(tv), scalar2=None,
                                            op0=EQ)

            def b_mask(out_ap, in_ap, lv, engine):
                # out = (in == lv), exact on the {0,2,3,5,9} EDG alphabet
                if engine == "act":
                    if lv == LMAX:
                        nc.scalar.activation(out_ap, in_ap, RELU,
                                             bias=bias_ap(1 - lv), scale=1.0)
                    elif lv == 0:
                        nc.scalar.activation(out_ap, in_ap, RELU,
                                             bias=bias_ap(1), scale=-1.0)
                    else:
                        raise ValueError(lv)
                else:
                    nc.vector.tensor_scalar(out=out_ap, in0=in_ap,
                                            scalar1=float(lv), scalar2=None,
                                            op0=EQ)

            # engine split: at most one A val and one B val are act-able;
            # put those on ACT, everything else on DVE.
            a_eng = ["act" if tv in (TMAX, 0) else "dve" for tv, _ in pairs]
            if a_eng[0] == "act" and a_eng[1] == "act":
                a_eng[1] = "dve"
            b_eng = ["act" if lv in (LMAX, 0) else "dve" for _, lv in pairs]
            if b_eng[0] == "act" and b_eng[1] == "act":
                b_eng[1] = "dve"

            # a tiles: one per chunk pair so deps are per-pair
            a_tiles = [a_pool.tile([128, 2, 2, 512], FP8, name=f"a{kk}",
                                   tag=f"a{kk}") for kk in range(KPAIRS)]

            def emit_phase1(kk):
                if kk == 0:
                    g = g_pre
                else:
                    g = g_pool.tile([128, 2, 2, P], FP8, tag="g")
                    nc.sync.dma_start(out=g[:], in_=G[kk])
                tp = tp_pool.tile([128, 2, 512], F32, tag="tp")
                for c in range(2):
                    ec = 2 * kk + c
                    nc.tensor.matmul(
                        tp[:, c, :],
                        lhsT=w_tiles[ec // wq][:, ec % wq, :, :],
                        rhs=g[:, c, :, :],
                        start=True, stop=True, perf_mode=DR)
                for q in range(2):
                    a_mask(a_tiles[kk][:, :, q, :], tp[:], pairs[q][0],
                           a_eng[q])

            b_live = {}

            def emit_bmask(nt, kk, lt_live, skip_dma=False):
                if kk % 2 == 0 and not skip_dma:
                    lt = lt_pool.tile([128, 4, 512], FP8, tag="lt")
                    nc.sync.dma_start(out=lt[:], in_=LT[nt, kk // 2])
                    lt_live[0] = lt
                lt = lt_live[0]
                cc = kk % 2
                b = b_pool.tile([128, 2, 2, 512], FP8, tag="b")
                for q in range(2):
                    # nt0 runs concurrently with phase 1, whose A masks
                    # occupy ACT -- keep B masks on DVE there, except in
                    # the pipeline-fill window where ACT is still idle
                    eng = b_eng[q] if (nt > 0 or kk < 4) else "dve"
                    b_mask(b[:, :, q, :], lt[:, 2 * cc:2 * cc + 2, :],
                           pairs[q][1], eng)
                b_live[(nt, kk)] = b

            def emit_p2(nt, kk, ep_tiles):
                b = b_live.pop((nt, kk))
                for c in range(2):
                    for m in range(MTILES):
                        nc.tensor.matmul(
                            ep_tiles[m],
                            lhsT=a_tiles[kk][:, c, :, m * 128:(m + 1) * 128],
                            rhs=b[:, c, :, :],
                            start=(kk == 0 and c == 0),
                            stop=(kk == KPAIRS - 1 and c == 1),
                            perf_mode=DR)

            def emit_outputs(nt, ep_tiles):
                # last pass: the epilogue is the kernel tail, so split it
                # across DVE (add) and ACT (plain copy; host adds null to
                # those two tiles during unshard)
                for m in range(MTILES):
                    ot = out_pool.tile([128, 512], F32, tag="out")
                    if nt == NTILES - 1 and m % 2 == 1:
                        nc.scalar.copy(out=ot[:], in_=ep_tiles[m])
                    else:
                        nc.vector.tensor_tensor(
                            out=ot[:], in0=ep_tiles[m],
                            in1=nb_sb[:, nt * 512:(nt + 1) * 512],
                            op=mybir.AluOpType.add)
                    nc.sync.dma_start(
                        out=en[m * 128:(m + 1) * 128,
                               nt * 512:(nt + 1) * 512],
                        in_=ot[:])

            # ---- nt0 pass, software-pipelined with phase 1 ----
            ep_tiles = [ep_pool.tile([128, 512], F32, name=f"ep0_{m}",
                                     tag="ep") for m in range(MTILES)]
            lt_live = [None]
            for kk in range(KPAIRS + LAG):
                if kk < KPAIRS:
                    if kk in (0, 2):
                        lt_live[0] = lt_pre[kk // 2]
                    emit_bmask(0, kk, lt_live, skip_dma=(kk in (0, 2)))
                    emit_phase1(kk)
                    if kk % 4 == 1 and kk // 4 < 7:
                        emit_w_dma(kk // 4 + 1)
                    if kk == 8:
                        nc.sync.dma_start(out=nb_sb[:], in_=NB[:])
                if kk >= LAG:
                    emit_p2(0, kk - LAG, ep_tiles)
            emit_outputs(0, ep_tiles)

            # ---- nt1..3 passes ----
            for nt in range(1, NTILES):
                ep_tiles = [ep_pool.tile([128, 512], F32,
                                         name=f"ep{nt}_{m}", tag="ep")
                            for m in range(MTILES)]
                lt_live = [None]
                for kk in range(KPAIRS + LAG):
                    if kk < KPAIRS:
                        emit_bmask(nt, kk, lt_live)
                    if kk >= LAG:
                        emit_p2(nt, kk - LAG, ep_tiles)
                emit_outputs(nt, ep_tiles)
    if not nc.is_finalized():
        nc.finalize()
    return nc


def _get_nc(pairs):
    key = tuple(pairs)
    if key not in _nc_cache:
        _nc_cache[key] = _build_nc(key)
    return _nc_cache[key]


def _prep_inputs(node_activations, learned_edge_states, edge_endnode_idx):
    na = np.asarray(node_activations)
    L = np.asarray(learned_edge_states, dtype=np.float32)
    idx = np.asarray(edge_endnode_idx)

    # per-chunk node gather + one-hot weights
    idxc = idx.reshape(ECHUNKS, 128, 2)
    U = np.zeros((ECHUNKS, 128, 2), dtype=np.int64)
    Wt = np.zeros((128, ECHUNKS, 2, 128), dtype=NP_FP8)
    ar = np.arange(128)
    for ec in range(ECHUNKS):
        for s in range(2):
            u, pos = np.unique(idxc[ec, :, s], return_inverse=True)
            U[ec, :len(u), s] = u
            Wt[pos, ec, s, ar] = 2.0 if s == 0 else 1.0
    W8 = np.ascontiguousarray(Wt.reshape(128, ECHUNKS * 2 * 128))

    naT = np.ascontiguousarray(na.T).astype(NP_FP8)        # [nodes, pts]
    Gfull = naT[U]                                         # [ec, 128, 2, npts]

    LTf = np.ascontiguousarray(L.T).astype(NP_FP8)         # [edges, cmp]
    null_count = (L == 0.0).sum(axis=1).astype(np.float32)  # [cmp]
    nb_full = np.broadcast_to(null_count[None, :],
                              (128, N_CMP)).astype(np.float32)

    in_maps = []
    for pg in range(PGROUPS):
        # G tiled: [kk, k, (c, side, p)]
        Gp = (Gfull[:, :, :, pg * P:(pg + 1) * P]
              .reshape(KPAIRS, 2, 128, 2, P)
              .transpose(0, 2, 1, 3, 4)
              .reshape(KPAIRS, 128, 2 * 2 * P))
        Gp = np.ascontiguousarray(Gp)
        for cg in range(CGROUPS):
            # LT tiled: [nt, q, ki, (c4, j)]
            lt = (LTf[:, cg * C:(cg + 1) * C]
                  .reshape(QUADS, 4, 128, NTILES, 512)
                  .transpose(3, 0, 2, 1, 4)
                  .reshape(NTILES, QUADS, 128, 4 * 512))
            in_maps.append({
                "W": W8,
                "G": Gp,
                "LT": np.ascontiguousarray(lt),
                "NB": np.ascontiguousarray(nb_full[:, cg * C:(cg + 1) * C]),
            })
    return in_maps


def _kept_pairs(edge_type_filter):
    seen = []
    for v in np.asarray(edge_type_filter).ravel().tolist():
        v = int(v)
        if v in _CODE2TEMP and v not in [p[1] for p in seen]:
            seen.append((_CODE2TEMP[v], v))
    return tuple(seen)


def _host_fallback(node_activations, learned_edge_states, edge_endnode_idx,
                   pairs):
    # exact host path for filter shapes the device kernel doesn't cover
    na = np.asarray(node_activations)
    L = np.asarray(learned_edge_states, dtype=np.float32)
    idx = np.asarray(edge_endnode_idx)
    temp = na[:, idx[:, 0]] * 2 + na[:, idx[:, 1]]
    en = np.broadcast_to((L == 0.0).sum(axis=1).astype(np.float32)[None, :],
                         (na.shape[0], L.shape[0])).copy()
    for tv, lv in pairs:
        a = (temp == tv).astype(np.float32)
        b = (L == float(lv)).astype(np.float32)
        en += a @ b.T
    return en - en.min()


def kernel(node_activations, learned_edge_states, edge_endnode_idx,
           edge_type_filter, _trace=False, _tmpdir=None):
    pairs = _kept_pairs(edge_type_filter)
    if len(pairs) != 2:
        return _host_fallback(node_activations, learned_edge_states,
                              edge_endnode_idx, pairs)

    nc = _get_nc(pairs)
    in_maps = _prep_inputs(node_activations, learned_edge_states,
                           edge_endnode_idx)
    res = run_bass_kernel_spmd(nc, in_maps, core_ids=list(range(8)),
                               trace=_trace, tmpdir=_tmpdir)
    L = np.asarray(learned_edge_states, dtype=np.float32)
    null_count = (L == 0.0).sum(axis=1).astype(np.float32)
    out = np.empty((N_PTS, N_CMP), dtype=np.float32)
    for ci in range(8):
        pg, cg = ci // CGROUPS, ci % CGROUPS
        blk = np.array(res.results[ci]["en"])
        # device skipped the null add on the last cmp tile's odd m rows
        nt, c0 = NTILES - 1, cg * C
        for m in (1, 3):
            blk[m * 128:(m + 1) * 128, nt * 512:(nt + 1) * 512] += \
                null_count[None, c0 + nt * 512:c0 + (nt + 1) * 512]
        out[pg * P:(pg + 1) * P, cg * C:(cg + 1) * C] = blk
    out -= out.min()
    if _trace:
        kernel._last_results = res
    return out
